# revision 8
# baseline (speedup 1.0000x reference)
"""VQ codebook decoder on 8 Trainium2 NeuronCores.

Strategy: data-parallel over tokens, but tokens are assigned to cores in
globally index-sorted order. Each core then deduplicates its 2304 tokens to
at most U_PAD=1024 unique codebook entries (seed-independent bound checked at
runtime, with a dense fallback):

  Phase A: indirect-DMA gather the core's unique codebook rows (bf16,
    transposed layout straight from the gather), run the 2-layer MLP
    (1024 -> 4096 gelu -> 1024) in bf16 on the tensor engine with fp32
    PSUM accumulation, write decoded rows to a DRAM table.
  Phase B: per 128-token block, indirect-DMA gather decoded rows by each
    token's dedup rank (f32) and DMA to the output. Blocks only read the
    prefix of the decoded table they need (ranks of sorted tokens are
    monotone), so Phase B overlaps Phase A.

The host applies the inverse token permutation when unsharding. mm1
produces h transposed ([H-part, tok]) so mm2 contracts over H without any
on-chip transpose; gelu+bias ride the scalar-engine PSUM eviction, the
output bias rides the vector-engine eviction.
"""

import sys

if "/opt/trn_rl_repo" not in sys.path:
    sys.path.insert(0, "/opt/trn_rl_repo")

import numpy as np
import ml_dtypes

import concourse.bass as bass
import concourse.mybir as mybir
import concourse.tile as tile
from concourse import bacc
from concourse.bass_utils import run_bass_kernel_spmd

B, M = 32, 576
CB, D, H, O = 8192, 1024, 4096, 1024
N_CORES = 8
T_TOTAL = B * M          # 18432
T = T_TOTAL // N_CORES   # 2304 tokens per core
P = 128
DK = D // P              # 8  k-subtiles for mm1
HK = H // P              # 32 k-subtiles for mm2
NO = O // 512            # 2  output column halves

U_PAD = 1024             # max unique codebook rows per core (dedup path)
US = 256                 # unique-slice size (Phase A granularity)
NU = U_PAD // US         # 4 unique slices
TB = 128                 # token block size (Phase B granularity)
NTB = T // TB            # 18 token blocks

BF16 = mybir.dt.bfloat16
F32 = mybir.dt.float32

_cache: dict = {}


def _wrap16(v):
    """int16 index layout for dma_gather: token j at [j%16, j//16], the
    16-row block replicated 8x down the 128 partitions."""
    v = np.asarray(v).astype(np.int16)
    return np.ascontiguousarray(np.tile(v.reshape(-1, 16).T, (8, 1)))


def _mlp_slice(nc, w1sb, w2sb, b1sb, b2sb, qpool, hpool, opool, p1pool,
               p2pool, cb16, idxsb, idx_col0, n_tok, store):
    """Gather n_tok codebook rows (by idx cols starting at idx_col0) and run
    the MLP; store(t2, osb) consumes each 128-row fp32 output block."""
    qT = qpool.tile([P, DK, n_tok], BF16, name="qT")
    nc.gpsimd.dma_gather(
        qT[:, :, :], cb16[:, :],
        idxsb[:, idx_col0:idx_col0 + n_tok // 16],
        n_tok, n_tok, D, transpose=True,
    )
    hT = hpool.tile([P, HK, n_tok], BF16, name="hT")
    for h in range(HK):
        ps1 = p1pool.tile([P, n_tok], F32, name="ps1")
        for ks in range(DK):
            nc.tensor.matmul(
                ps1[:, :],
                w1sb[:, ks, h * P:(h + 1) * P],
                qT[:, ks, :],
                start=(ks == 0), stop=(ks == DK - 1),
            )
        nc.scalar.activation(
            hT[:, h, :], ps1[:, :],
            mybir.ActivationFunctionType.Gelu_apprx_tanh,
            bias=b1sb[:, h:h + 1],
        )
    for t2 in range(n_tok // P):
        osb = opool.tile([P, O], F32, name="osb")
        for o in range(NO):
            ps2 = p2pool.tile([P, 512], F32, name="ps2")
            for ks in range(HK):
                nc.tensor.matmul(
                    ps2[:, :],
                    hT[:, ks, t2 * P:(t2 + 1) * P],
                    w2sb[:, ks, o * 512:(o + 1) * 512],
                    start=(ks == 0), stop=(ks == HK - 1),
                )
            nc.vector.tensor_add(
                osb[:, o * 512:(o + 1) * 512], ps2[:, :],
                b2sb[:, o * 512:(o + 1) * 512],
            )
        store(t2, osb)


def _declare_common(nc):
    cb16 = nc.declare_dram_parameter("cb16", [CB, D], BF16, isOutput=False)
    w1 = nc.declare_dram_parameter("w1", [D, H], BF16, isOutput=False)
    w2 = nc.declare_dram_parameter("w2", [H, O], BF16, isOutput=False)
    b1r = nc.declare_dram_parameter("b1r", [P, HK], F32, isOutput=False)
    b2r = nc.declare_dram_parameter("b2r", [P, O], F32, isOutput=False)
    return cb16, w1, w2, b1r, b2r


def _load_weights(nc, wpool, w1, w2, b1r, b2r):
    b1sb = wpool.tile([P, HK], F32)
    nc.sync.dma_start(out=b1sb[:], in_=b1r[:])
    # w1 split by h-column ranges to match mm1's consumption order (h-major),
    # so the first matmuls can start as soon as the first chunk lands.
    # w1sb[p, ks, h] = W1[ks*128+p, h]
    w1sb = wpool.tile([P, DK, H], BF16)
    w1v = w1.rearrange("(ks p) h -> p ks h", p=P)
    HC = 512
    for h0 in range(0, H, HC):
        nc.sync.dma_start(out=w1sb[:, :, h0:h0 + HC], in_=w1v[:, :, h0:h0 + HC])
    b2sb = wpool.tile([P, O], F32)
    nc.sync.dma_start(out=b2sb[:], in_=b2r[:])
    w2sb = wpool.tile([P, HK, O], BF16)
    w2v = w2.rearrange("(ks p) o -> p ks o", p=P)
    for ks in range(0, HK, 8):
        nc.sync.dma_start(out=w2sb[:, ks:ks + 8, :], in_=w2v[:, ks:ks + 8, :])
    return w1sb, w2sb, b1sb, b2sb


def _build_dedup(schedule, repeats: int = 1):
    """schedule[i] = number of 256-row dec slices token block i needs."""
    nc = bacc.Bacc("TRN2", target_bir_lowering=False, debug=False,
                   num_devices=N_CORES)
    cb16, w1, w2, b1r, b2r = _declare_common(nc)
    uidx16 = nc.declare_dram_parameter("uidx16", [P, U_PAD // 16],
                                       mybir.dt.int16, isOutput=False)
    rank16 = nc.declare_dram_parameter("rank16", [P, T // 16],
                                       mybir.dt.int16, isOutput=False)
    out = nc.declare_dram_parameter("out", [T, O], F32, isOutput=True)

    with tile.TileContext(nc) as tc:
        with (
            tc.tile_pool(name="wpool", bufs=1) as wpool,
            tc.tile_pool(name="qpool", bufs=2) as qpool,
            tc.tile_pool(name="hpool", bufs=2) as hpool,
            tc.tile_pool(name="opool", bufs=2) as opool,
            tc.tile_pool(name="g2pool", bufs=2) as g2pool,
            tc.tile_pool(name="dpool", bufs=1, space="DRAM") as dpool,
            tc.tile_pool(name="p1pool", bufs=4, space="PSUM") as p1pool,
            tc.tile_pool(name="p2pool", bufs=2, space="PSUM") as p2pool,
        ):
          for _rep in range(repeats):
            uidxsb = wpool.tile([P, U_PAD // 16], mybir.dt.int16)
            nc.sync.dma_start(out=uidxsb[:], in_=uidx16[:])
            w1sb, w2sb, b1sb, b2sb = _load_weights(nc, wpool, w1, w2, b1r, b2r)
            ranksb = wpool.tile([P, T // 16], mybir.dt.int16)
            nc.sync.dma_start(out=ranksb[:], in_=rank16[:])
            dec = dpool.tile([U_PAD, O], F32)

            def emit_tok_block(i):
                # schedule[i] counts 128-row dec blocks this token block needs
                need = P * schedule[i]
                g2 = g2pool.tile([P, 1, O], F32, name="g2")
                nc.gpsimd.dma_gather(
                    g2[:, :, :], dec[0:need, :],
                    ranksb[:, i * (TB // 16):(i + 1) * (TB // 16)],
                    TB, TB, O,
                )
                nc.sync.dma_start(out=out[i * TB:(i + 1) * TB, :],
                                  in_=g2[:, 0, :])

            state = {"emitted": 0, "dec_done": 0}

            def after_store():
                state["dec_done"] += 1
                while (state["emitted"] < NTB
                       and schedule[state["emitted"]] <= state["dec_done"]):
                    emit_tok_block(state["emitted"])
                    state["emitted"] += 1

            for j in range(NU):
                def store(t2, osb, j=j):
                    row = j * US + t2 * P
                    nc.sync.dma_start(out=dec[row:row + P, :], in_=osb[:])
                    after_store()
                _mlp_slice(nc, w1sb, w2sb, b1sb, b2sb, qpool, hpool, opool,
                           p1pool, p2pool, cb16, uidxsb, j * (US // 16), US,
                           store)
            while state["emitted"] < NTB:
                emit_tok_block(state["emitted"])
                state["emitted"] += 1

    nc.compile()
    return nc


def _build_dense(repeats: int = 1):
    """Fallback: straight data-parallel, no dedup (2304 tokens per core)."""
    nc = bacc.Bacc("TRN2", target_bir_lowering=False, debug=False,
                   num_devices=N_CORES)
    cb16, w1, w2, b1r, b2r = _declare_common(nc)
    idx16 = nc.declare_dram_parameter("idx16", [P, T // 16], mybir.dt.int16,
                                      isOutput=False)
    out = nc.declare_dram_parameter("out", [T, O], F32, isOutput=True)
    TS = 256

    with tile.TileContext(nc) as tc:
        with (
            tc.tile_pool(name="wpool", bufs=1) as wpool,
            tc.tile_pool(name="qpool", bufs=2) as qpool,
            tc.tile_pool(name="hpool", bufs=2) as hpool,
            tc.tile_pool(name="opool", bufs=3) as opool,
            tc.tile_pool(name="p1pool", bufs=4, space="PSUM") as p1pool,
            tc.tile_pool(name="p2pool", bufs=2, space="PSUM") as p2pool,
        ):
          for _rep in range(repeats):
            idxsb = wpool.tile([P, T // 16], mybir.dt.int16)
            nc.sync.dma_start(out=idxsb[:], in_=idx16[:])
            w1sb, w2sb, b1sb, b2sb = _load_weights(nc, wpool, w1, w2, b1r, b2r)
            for i in range(T // TS):
                def store(t2, osb, i=i):
                    row = i * TS + t2 * P
                    nc.sync.dma_start(out=out[row:row + P, :], in_=osb[:])
                _mlp_slice(nc, w1sb, w2sb, b1sb, b2sb, qpool, hpool, opool,
                           p1pool, p2pool, cb16, idxsb, i * (TS // 16), TS,
                           store)

    nc.compile()
    return nc


def _get_nc(kind, schedule=None, repeats=1):
    key = (kind, schedule, repeats)
    if key not in _cache:
        if kind == "dedup":
            _cache[key] = _build_dedup(schedule, repeats)
        else:
            _cache[key] = _build_dense(repeats)
    return _cache[key]


def _prep_weights(codebook, W1, b1, W2, b2):
    bf = ml_dtypes.bfloat16
    return {
        "cb16": np.ascontiguousarray(codebook.astype(bf)),
        "w1": np.ascontiguousarray(W1.astype(bf)),
        "w2": np.ascontiguousarray(W2.astype(bf)),
        "b1r": np.ascontiguousarray(b1.astype(np.float32).reshape(HK, P).T),
        "b2r": np.ascontiguousarray(
            np.broadcast_to(b2.astype(np.float32)[None, :], (P, O))),
    }


def _plan_dedup(index):
    """Sorted-index sharding + per-core dedup. Returns None if any core
    exceeds U_PAD unique rows (caller falls back to the dense kernel)."""
    idx_flat = np.asarray(index).reshape(-1)
    order = np.argsort(idx_flat, kind="stable")
    perms, uidxs, ranks, needs = [], [], [], []
    for c in range(N_CORES):
        perm = order[c * T:(c + 1) * T]
        vals = idx_flat[perm]
        uniq, inv = np.unique(vals, return_inverse=True)
        if uniq.size > U_PAD:
            return None
        up = np.zeros(U_PAD, np.int64)
        up[:uniq.size] = uniq
        perms.append(perm)
        uidxs.append(_wrap16(up))
        ranks.append(_wrap16(inv))
        # 128-row dec block count needed by each 128-token block of this core
        need = [int(np.ceil((inv[i * TB:(i + 1) * TB].max() + 1) / P))
                for i in range(NTB)]
        needs.append(need)
    schedule = tuple(max(needs[c][i] for c in range(N_CORES))
                     for i in range(NTB))
    return perms, uidxs, ranks, schedule


def kernel(index, codebook, W1, b1, W2, b2):
    wmaps = _prep_weights(codebook, W1, b1, W2, b2)
    plan = _plan_dedup(index)
    if plan is not None:
        perms, uidxs, ranks, schedule = plan
        nc = _get_nc("dedup", schedule)
        in_maps = [{**wmaps, "uidx16": uidxs[c], "rank16": ranks[c]}
                   for c in range(N_CORES)]
        res = run_bass_kernel_spmd(nc, in_maps, list(range(N_CORES)))
        out = np.empty((T_TOTAL, O), np.float32)
        for c in range(N_CORES):
            out[perms[c]] = res.results[c]["out"]
    else:
        nc = _get_nc("dense")
        idx_flat = np.asarray(index).reshape(-1)
        in_maps = [{**wmaps, "idx16": _wrap16(idx_flat[c * T:(c + 1) * T])}
                   for c in range(N_CORES)]
        res = run_bass_kernel_spmd(nc, in_maps, list(range(N_CORES)))
        out = np.concatenate([res.results[c]["out"] for c in range(N_CORES)],
                             axis=0)
    return out.reshape(B, M, O).astype(np.float32)
